# revision 25
# baseline (speedup 1.0000x reference)
"""Distance-weighted self-attention on 8 Trainium2 NeuronCores.

The reference network is rank-1 in d_model:
  q = h*Wq, k = h*Wk, v = h*Wv  (h = heights column)
  logits[p,k] = c*h_p*h_k - 0.5*|sig_p - sig_k|,  c = (Wq.Wk)/sqrt(256)
  out[p,:]   = (sum_k softmax(logits)[p,k]*h_k) * Wv.

Key identity used here: with L(p) = {k : sig_k <= sig_p},
  exp(-0.5|sig_p - sig_k|) = e^{-0.5 sig_p} e^{+0.5 sig_k}   for k in L(p)
                           = e^{+0.5 sig_p} e^{-0.5 sig_k}   otherwise,
and since |c*h_p*h_k| <~ 0.05, exp(c h_p h_k) = 1 + c h_p h_k to ~1e-3.
With the +-1/2 step convention s(p,k) = [sig_k <= sig_p] - 1/2 (ties -> 0,
exact because both branch formulas agree at sig_k == sig_p), the masked
sums A_m(p) = sum_k s(p,k) g_k h_k^m obey A_m = P_m + T_m/2 with P from
the sign-mask matmuls and T the plain totals.  Dividing num/den by
e^{-0.5 sig_p} (cancels in the ratio):
  den'_p = Q0 + a*Q1 + E*su2 + g1,  Q_m = P_m - E*P_{m+4}, E = e^{sig_p}
  num'_p = Q1 + a*Q2 + E*sup2 + g2
so the only O(S^2) device work is one 4x-mode DVE compare per key-chunk
half (3 chunks use +-1 Sign masks on the scalar engine) and tiny PE
matmuls lhsT=mask[128,128] x rhs=moments[128,8] accumulated into a
[128,8]-per-query-chunk PSUM bank (matmul start=True resets a whole PSUM
bank, so each half owns one bank and only its first matmul sets start).

The output outer product a x Wv runs as per-partition-scalar multiplies
into fp16, written back per quarter (host upcasts to f32), so the 1 MB
writeback overlaps the second half's mask phase.  Engines execute their
streams strictly in order, so emission order is laid out explicitly: the
scalar engine interleaves its PSUM->SBUF copy between the two sign-mask
halves, and the vector engine interleaves the half-0 combine into the
half-1 mask stream.
"""

import os
import sys

import numpy as np

for _p in ("/opt/trn_rl_repo", "/root/.axon_site/_ro/trn_rl_repo"):
    if os.path.isdir(_p) and _p not in sys.path:
        sys.path.append(_p)

import concourse.bacc as bacc
import concourse.bass as bass
import concourse.mybir as mybir
import concourse.tile as tile
from concourse.bass_utils import run_bass_kernel_spmd

S = 2048
D = 256
P = 128
NJ = S // P  # 16
N_CORES = 8
HALF = S // 2

f32 = mybir.dt.float32
f16 = mybir.dt.float16
Alu = mybir.AluOpType
Act = mybir.ActivationFunctionType

ACT_JS = (13, 14, 15)  # key chunks whose masks run on the scalar engine
POOL_H1_JS = (11, 12)  # key chunks whose half-1 masks run on gpsimd
DVE_JS = tuple(j for j in range(NJ) if j not in ACT_JS)
DVE_H1_JS = tuple(j for j in DVE_JS if j not in POOL_H1_JS)


def build_kernel(nc: bass.Bass):
    x = nc.dram_tensor("x", [2, S], f16, kind="ExternalInput").ap()
    # host-pretransposed columns: xc[p, c*NJ + j] = x[c, j*128 + p]
    xc = nc.dram_tensor("xc", [P, 2 * NJ], f16, kind="ExternalInput").ap()
    # Wq/Wk in column layout [128, 2] (host-reshaped) for the PE c-reduction
    wq = nc.dram_tensor("wq", [P, 2], f32, kind="ExternalInput").ap()
    wk = nc.dram_tensor("wk", [P, 2], f32, kind="ExternalInput").ap()
    wv16 = nc.dram_tensor("wv16", [1, D], f16, kind="ExternalInput").ap()
    out = nc.dram_tensor("out", [S, D], f16, kind="ExternalOutput").ap()

    with tile.TileContext(nc) as tc:
        from contextlib import ExitStack

        with ExitStack() as ctx:
            const = ctx.enter_context(tc.tile_pool(name="const", bufs=1))
            mpool = ctx.enter_context(tc.tile_pool(name="mpool", bufs=16))
            apsum = ctx.enter_context(
                tc.tile_pool(name="apsum", bufs=1, space=bass.MemorySpace.PSUM)
            )
            tpsum = ctx.enter_context(
                tc.tile_pool(name="tpsum", bufs=1, space=bass.MemorySpace.PSUM)
            )
            _body(nc, tc, const, mpool, apsum, tpsum, x, xc, wq, wk, wv16, out)
    return nc


def _body(nc, tc, const, mpool, apsum, tpsum, x, xc, wq, wk, wv16, out):
    # ---- DMAs: sig_rep half 0 gates the first masks — it goes first -----
    sig_rep = const.tile([P, S], f16)
    nc.sync.dma_start(sig_rep[:, 0:HALF], x[0:1, 0:HALF].to_broadcast([P, HALF]))
    colq = const.tile([P, 2 * NJ], f16)
    nc.scalar.dma_start(colq[:], xc)
    nc.sync.dma_start(sig_rep[:, HALF:S], x[0:1, HALF:S].to_broadcast([P, HALF]))
    wv_rep = const.tile([P, D], f16)
    nc.sync.dma_start(wv_rep[:], wv16.to_broadcast([P, D]))
    wq_t = const.tile([P, 2], f32)
    wk_t = const.tile([P, 2], f32)
    nc.sync.dma_start(wq_t[:], wq)
    nc.sync.dma_start(wk_t[:], wk)

    # ---- ACT: act-table preload during the DMA window -------------------
    dummy = const.tile([1, 1], f32)
    nc.vector.memset(dummy[:], 0.0)
    nc.scalar.activation(dummy[:], dummy[:], Act.Sign)

    # ---- DVE prologue: columns then straight into masks -----------------
    ones = const.tile([P, P], f16)
    nc.vector.memset(ones[:], 1.0)
    colf = const.tile([P, 2 * NJ], f32)
    nc.vector.tensor_copy(colf[:], colq[:])
    sig_col = colf[:, 0:NJ]
    h_col = colf[:, NJ : 2 * NJ]
    negsig = const.tile([P, NJ], f32)
    nc.vector.tensor_scalar_mul(negsig[:], sig_col, -1.0)

    gp = const.tile([P, NJ], f32)
    gm = const.tile([P, NJ], f32)
    ecol = const.tile([P, NJ], f32)
    nc.scalar.activation(gp[:], sig_col, Act.Exp, scale=0.5)
    nc.scalar.activation(gm[:], sig_col, Act.Exp, scale=-0.5)

    # ---- Pool prologue: moments, c-chain --------------------------------
    mom = const.tile([P, 8 * NJ], f16)
    nc.gpsimd.memset(mom[:], 0.0)
    h2 = const.tile([P, NJ], f32)
    nc.gpsimd.tensor_mul(h2[:], h_col, h_col)
    h16 = const.tile([P, NJ], f32)  # h/16 so a = (h/16) * (Wq.Wk)
    nc.gpsimd.tensor_scalar_mul(h16[:], h_col, 1.0 / 16.0)
    momv = mom[:].rearrange("p (j m) -> p j m", m=8)
    nc.gpsimd.tensor_copy(momv[:, :, 0], gp[:])
    nc.gpsimd.tensor_mul(momv[:, :, 1], gp[:], h_col)
    nc.gpsimd.tensor_mul(momv[:, :, 2], gp[:], h2[:])
    nc.gpsimd.tensor_copy(momv[:, :, 4], gm[:])
    nc.gpsimd.tensor_mul(momv[:, :, 5], gm[:], h_col)
    nc.gpsimd.tensor_mul(momv[:, :, 6], gm[:], h2[:])
    momh3 = const.tile([P, 8 * len(ACT_JS)], f16)
    nc.gpsimd.tensor_scalar_mul(momh3[:], mom[:, 8 * ACT_JS[0] : 8 * (ACT_JS[-1] + 1)], 0.5)
    nc.gpsimd.tensor_mul(ecol[:], gp[:], gp[:])  # E = e^{sig} = gp^2
    wqkc = const.tile([P, 2], f16)
    nc.gpsimd.tensor_mul(wqkc[:], wq_t[:], wk_t[:])

    # ---- PE: totals, then the c-reduction -------------------------------
    psum_t = tpsum.tile([P, 7], f32, tag="pt")
    for j in range(NJ):
        nc.tensor.matmul(
            psum_t[:],
            ones[:],
            mom[:, 8 * j : 8 * j + 7],
            start=(j == 0),
            stop=(j == NJ - 1),
            skip_group_check=True,
        )
    psum_c = tpsum.tile([P, 2], f32, tag="pc")
    nc.tensor.matmul(psum_c[:], ones[:], wqkc[:], start=True, stop=True, skip_group_check=True)

    # ---- Pool: per-query globals (t2/c_red filled by DVE post-h0-masks) -
    t2 = const.tile([P, 7], f32)  # T_m / 2
    c_col = const.tile([P, 1], f32)
    a_col = const.tile([P, NJ], f32)
    su2 = const.tile([P, NJ], f32)  # (T4 + a*T5)/2
    sup2 = const.tile([P, NJ], f32)  # (T5 + a*T6)/2
    g1 = const.tile([P, NJ], f32)  # (T0 + a*T1)/2
    g2 = const.tile([P, NJ], f32)  # (T1 + a*T2)/2
    esu = const.tile([P, NJ], f32)
    esup = const.tile([P, NJ], f32)

    def pool_globals():
        nc.gpsimd.tensor_scalar_mul(a_col[:], h16[:], c_col[:])
        nc.gpsimd.tensor_scalar(su2[:], a_col[:], t2[:, 5:6], t2[:, 4:5], op0=Alu.mult, op1=Alu.add)
        nc.gpsimd.tensor_scalar(sup2[:], a_col[:], t2[:, 6:7], t2[:, 5:6], op0=Alu.mult, op1=Alu.add)
        nc.gpsimd.tensor_scalar(g1[:], a_col[:], t2[:, 1:2], t2[:, 0:1], op0=Alu.mult, op1=Alu.add)
        nc.gpsimd.tensor_scalar(g2[:], a_col[:], t2[:, 2:3], t2[:, 1:2], op0=Alu.mult, op1=Alu.add)
        nc.gpsimd.tensor_mul(esu[:], ecol[:], su2[:])
        nc.gpsimd.tensor_add(esu[:], esu[:], g1[:])
        nc.gpsimd.tensor_mul(esup[:], ecol[:], sup2[:])
        nc.gpsimd.tensor_add(esup[:], esup[:], g2[:])

    # ---- masks and matmuls ----------------------------------------------
    sgn = {}
    for j in ACT_JS:
        sgn[j] = const.tile([P, S], f16, name=f"sgn{j}", tag=f"sgn{j}")
    psum_a = {}
    mstate = {}
    for h in range(2):
        psum_a[h] = apsum.tile([P, 64], f32, tag=f"pa{h}", name=f"pa{h}")
        mstate[h] = {"first": True}

    def act_sgn_half(h):
        lo, hi = HALF * h, HALF * (h + 1)
        for j in ACT_JS:
            nc.scalar.activation(
                sgn[j][:, lo:hi], sig_rep[:, lo:hi], Act.Sign, bias=negsig[:, j : j + 1]
            )

    def mask_produce(h, j, eng=None, tag="mask"):
        lo = HALF * h
        m = mpool.tile([P, HALF], f16, tag=tag, name=f"m{tag}{h}{j}")
        (eng or nc.vector).tensor_scalar(
            m[:],
            sig_rep[:, lo : lo + HALF],
            sig_col[:, j : j + 1],
            0.5,
            op0=Alu.is_ge,
            op1=Alu.subtract,
        )
        return m

    def mask_matmuls(h, j, m, stop=False):
        st = mstate[h]
        for il in range(8):
            nc.tensor.matmul(
                psum_a[h][:, 8 * il : 8 * il + 8],
                m[:, P * il : P * (il + 1)],
                mom[:, 8 * j : 8 * j + 8],
                start=st["first"],
                stop=stop and il == 7,
                skip_group_check=True,
            )
            st["first"] = False

    def dve_mask(h, j, eng=None):
        mask_matmuls(h, j, mask_produce(h, j, eng=eng))

    def act_matmuls(h, with_stop=True):
        lo = HALF * h
        for jx, j in enumerate(ACT_JS):
            for il in range(8):
                last = with_stop and jx == len(ACT_JS) - 1 and il == 7
                nc.tensor.matmul(
                    psum_a[h][:, 8 * il : 8 * il + 8],
                    sgn[j][:, lo + P * il : lo + P * (il + 1)],
                    momh3[:, 8 * jx : 8 * jx + 8],
                    start=False,
                    stop=last,
                    skip_group_check=True,
                )

    out_r = out.rearrange("(i p) d -> p i d", p=P)
    comb = {}

    def acopy(h):
        acp = const.tile([P, 64], f32, name=f"acp{h}", tag=f"acp{h}")
        nc.scalar.copy(acp[:], psum_a[h][:])
        comb[h] = {"acp": acp}

    def pool_combine(h):
        acp = comb[h]["acp"]
        A = acp[:].rearrange("p (i m) -> p m i", m=8)
        cs = slice(8 * h, 8 * (h + 1))
        eh = ecol[:, cs]

        def ptt(name, in0, in1, op):
            t = const.tile([P, 8], f32, name=name + str(h), tag=name + str(h))
            nc.gpsimd.tensor_tensor(t[:], in0, in1, op=op)
            return t

        w0 = ptt("w0", eh, A[:, 4], Alu.mult)
        w1 = ptt("w1", eh, A[:, 5], Alu.mult)
        w2 = ptt("w2", eh, A[:, 6], Alu.mult)
        comb[h]["q0"] = ptt("q0", A[:, 0], w0[:], Alu.subtract)
        comb[h]["q1"] = ptt("q1", A[:, 1], w1[:], Alu.subtract)
        comb[h]["q2"] = ptt("q2", A[:, 2], w2[:], Alu.subtract)

    def dve_combine(h):
        cs = slice(8 * h, 8 * (h + 1))
        ah = a_col[:, cs]

        def vtt(name, in0, in1, op):
            t = const.tile([P, 8], f32, name=name + str(h), tag=name + str(h))
            nc.vector.tensor_tensor(t[:], in0, in1, op=op)
            return t

        if "q0" not in comb[h]:
            acp = comb[h]["acp"]
            A = acp[:].rearrange("p (i m) -> p m i", m=8)
            eh = ecol[:, cs]
            w0 = vtt("w0", eh, A[:, 4], Alu.mult)
            w1 = vtt("w1", eh, A[:, 5], Alu.mult)
            w2 = vtt("w2", eh, A[:, 6], Alu.mult)
            comb[h]["q0"] = vtt("q0", A[:, 0], w0[:], Alu.subtract)
            comb[h]["q1"] = vtt("q1", A[:, 1], w1[:], Alu.subtract)
            comb[h]["q2"] = vtt("q2", A[:, 2], w2[:], Alu.subtract)
        q0, q1, q2 = comb[h]["q0"], comb[h]["q1"], comb[h]["q2"]

        u1 = vtt("u1", ah, q1[:], Alu.mult)
        d1 = vtt("d1", q0[:], u1[:], Alu.add)
        den = vtt("dn", d1[:], esu[:, cs], Alu.add)
        z1 = vtt("z1", ah, q2[:], Alu.mult)
        n1 = vtt("n1", q1[:], z1[:], Alu.add)
        num = vtt("nm", n1[:], esup[:, cs], Alu.add)
        inv = const.tile([P, 8], f32, name=f"inv{h}", tag=f"inv{h}")
        nc.vector.reciprocal_approx_fast(inv[:], den[:])
        comb[h]["aout"] = vtt("ao", num[:], inv[:], Alu.mult)

    def outers_quarter(qq, engines):
        h = qq // 2
        q = qq % 2
        aout = comb[h]["aout"]
        ob = const.tile([P, 4 * D], f16, name=f"ob{qq}", tag=f"ob{qq}")
        for il4 in range(4):
            sc = aout[:, 4 * q + il4 : 4 * q + il4 + 1]
            dst = ob[:, D * il4 : D * (il4 + 1)]
            engines[il4].tensor_scalar_mul(dst, wv_rep[:], sc)
        return ob

    def out_dma(qq, ob):
        nc.sync.dma_start(
            out_r[:, 4 * qq : 4 * (qq + 1)],
            ob[:].rearrange("p (i d) -> p i d", d=D),
        )

    OE = [nc.vector, nc.vector, nc.gpsimd, nc.gpsimd]

    OEV = [nc.vector] * 4

    # ---- emission ---------------------------------------------------------
    act_sgn_half(0)  # ACT: sgn h0 pieces (after exps)
    for j in DVE_JS:
        dve_mask(0, j)
    nc.vector.tensor_scalar(c_col[:], psum_c[:, 0:1], psum_c[:, 1:2], None, op0=Alu.add)
    act_matmuls(0)  # PE: ACT-mask matmuls h0 (incl stop)
    nc.scalar.mul(t2[:], psum_t[:], 0.5)  # ACT idle window
    acopy(0)  # ACT: psum->sbuf for h0 (before sgn h1 in ACT order)
    act_sgn_half(1)  # ACT: sgn h1 pieces

    # Pool: produce its two h1 masks early (own tile tag; matmuls deferred)
    pool_masks = {j: mask_produce(1, j, eng=nc.gpsimd, tag=f"pm{j}") for j in POOL_H1_JS}
    pool_globals()
    pool_combine(0)

    # DVE: uninterrupted h1 masks, then combine h0 + all early outputs
    for j in DVE_H1_JS:
        dve_mask(1, j)
    act_matmuls(1, with_stop=False)
    for jx, j in enumerate(POOL_H1_JS):
        mask_matmuls(1, j, pool_masks[j], stop=(jx == len(POOL_H1_JS) - 1))
    dve_combine(0)
    out_dma(0, outers_quarter(0, OEV))
    out_dma(1, outers_quarter(1, OEV))

    acopy(1)  # ACT (after sgn h1 in its stream)
    dve_combine(1)  # includes w/q on DVE for the short tail
    out_dma(2, outers_quarter(2, OEV))
    ob3 = outers_quarter(3, OEV)
    nc.sync.dma_start(out_r[:, 12:14], ob3[:, 0 : 2 * D].rearrange("p (i d) -> p i d", d=D))
    nc.scalar.dma_start(out_r[:, 14:16], ob3[:, 2 * D : 4 * D].rearrange("p (i d) -> p i d", d=D))


_NC = {}


def _get_nc():
    if "nc" not in _NC:
        nc = bacc.Bacc("TRN2", target_bir_lowering=False, debug=False, num_devices=N_CORES)
        build_kernel(nc)
        nc.compile()
        _NC["nc"] = nc
    return _NC["nc"]


def kernel(inputs: np.ndarray, Wq: np.ndarray, Wk: np.ndarray, Wv: np.ndarray) -> np.ndarray:
    assert inputs.shape == (N_CORES, S, 2), inputs.shape
    nc = _get_nc()
    wq = np.ascontiguousarray(np.asarray(Wq, np.float32).reshape(2, P).T)
    wk = np.ascontiguousarray(np.asarray(Wk, np.float32).reshape(2, P).T)
    wv16 = np.ascontiguousarray(np.asarray(Wv, dtype=np.float32).astype(np.float16))
    xs = [
        np.ascontiguousarray(np.asarray(inputs[b], dtype=np.float32).T.astype(np.float16))
        for b in range(N_CORES)
    ]
    # xc[p, c*NJ + j] = x[c, j*128 + p]
    xcs = [
        np.ascontiguousarray(xb.reshape(2, NJ, P).transpose(2, 0, 1).reshape(P, 2 * NJ))
        for xb in xs
    ]
    in_maps = [
        {"x": xs[b], "xc": xcs[b], "wq": wq, "wk": wk, "wv16": wv16}
        for b in range(N_CORES)
    ]
    res = run_bass_kernel_spmd(nc, in_maps, core_ids=list(range(N_CORES)))
    return np.stack([r["out"].astype(np.float32) for r in res.results], axis=0)
